# revision 14
# baseline (speedup 1.0000x reference)
"""Trainium2 distributed causal attention kernel (8 NeuronCores).

Problem: x[4,2048,1024] -> qkv proj -> 16-head causal attention -> out proj.

Sharding (uniform SPMD graph on all 8 cores):
  core c = (batch b = c//2, head-group g = c%2 of 8 heads).
  Each core: projects q/k/v for its 8 heads over the full 2048 tokens of its
  batch, runs causal flash-style attention (no max subtraction -- scores are
  O(1) for this input distribution), computes the partial output projection
  with its 512 inner dims of w_out, adds b_out/2, then a pairwise
  ReduceScatter(add) over {2b, 2b+1} yields final output token-stripes.
  Host reassembles stripes. No other collectives.

Layouts (all matmuls are layout-natural; x is transposed on the host):
  xT   [1024(dm), 2048(tok)]  f32r   (moving operand of kT/qT proj, stationary of v proj)
  kT,qT [512(inner) as 4x[128], 2048] bf16 (keys/queries transposed, 2 heads per tile)
  v_aug [2048(tok) as 16x[128], 8*65] bf16 (per head: 64 v-cols + ones col -> softmax denom)
  simT psum [128(key), 512(tok)] = k-block^T @ q-chunk   (K=64, heads packed 2x in PE array)
  pT = exp(simT * 0.125) bf16, causal band masks applied multiplicatively
  pv psum [65, 512] accumulates over k-blocks (row 64 = denominator)
  attnoutT bf16 [512(inner), 512(tok)] per chunk -> out-proj psum [128(tok), 512(col)]
"""

import sys

sys.path.insert(0, "/opt/trn_rl_repo")

import numpy as np

B, N, DM = 4, 2048, 1024
H, DH = 16, 64
HG = 8  # heads per core
LI = HG * DH  # local inner = 512
NCORES = 8
CHUNK = 512  # q-chunk tokens
NCHUNK = N // CHUNK  # 4
KB = 128  # k-block size
VW = DH + 1  # v columns per head incl. ones column

_GRAPH = None


def _build_graph(dbg=False):
    from concourse import bacc, bass, mybir, tile

    f32 = mybir.dt.float32
    f32r = mybir.dt.float32r
    bf16 = mybir.dt.bfloat16
    Exp = mybir.ActivationFunctionType.Exp

    nc = bacc.Bacc("TRN2", target_bir_lowering=False, debug=False)

    xT_d = nc.dram_tensor("xT", [DM, N], f32r, kind="ExternalInput")
    wq_d = nc.dram_tensor("wq", [DM, LI], f32r, kind="ExternalInput")
    wk_d = nc.dram_tensor("wk", [DM, LI], f32r, kind="ExternalInput")
    wv_d = nc.dram_tensor("wv", [DM, LI], f32r, kind="ExternalInput")
    wo_d = nc.dram_tensor("wo", [LI, DM], f32, kind="ExternalInput")
    hb_d = nc.dram_tensor("hb", [1, DM], f32, kind="ExternalInput")
    mask_d = nc.dram_tensor("mask", [KB, 4 * CHUNK], bf16, kind="ExternalInput")
    out_d = nc.dram_tensor("out", [N // 2, DM], f32, kind="ExternalOutput")
    if dbg:
        dkT = nc.dram_tensor("dkT", [128, N], f32, kind="ExternalOutput")
        dqT = nc.dram_tensor("dqT", [128, N], f32, kind="ExternalOutput")
        dva = nc.dram_tensor("dva", [128, HG * VW], f32, kind="ExternalOutput")
        dpt = nc.dram_tensor("dpt", [128, CHUNK], f32, kind="ExternalOutput")
        dpv = nc.dram_tensor("dpv", [VW, CHUNK], f32, kind="ExternalOutput")
        dao = nc.dram_tensor("dao", [128, CHUNK], f32, kind="ExternalOutput")
        dpd = nc.dram_tensor("dpd", [CHUNK, DM], f32, kind="ExternalOutput")

    RG = [[0, 1], [2, 3], [4, 5], [6, 7]]

    with tile.TileContext(nc) as tc:
        with (
            tc.tile_pool(name="persist", bufs=1) as pers,
            tc.tile_pool(name="projtmp", bufs=1) as ptmp,
            tc.tile_pool(name="wstream", bufs=3) as wstr,
            tc.tile_pool(name="work", bufs=4) as work,
            tc.tile_pool(name="aux", bufs=2) as aux,
            tc.tile_pool(name="mmps", bufs=2, space="PSUM") as mmps,
            tc.tile_pool(name="simps", bufs=4, space="PSUM") as simps,
            tc.tile_pool(name="pvps", bufs=2, space="PSUM") as pvps,
            tc.tile_pool(name="dram", bufs=2, space="DRAM") as dram,
        ):
            # ---- constants / persistent tiles ----
            mask_sb = pers.tile([KB, 4 * CHUNK], bf16, tag="mask")
            nc.sync.dma_start(out=mask_sb[:, :], in_=mask_d[:, :])

            ones_bf = pers.tile([1, KB], bf16, tag="ones")
            nc.vector.memset(ones_bf[:, :], 1.0)

            hb_f = aux.tile([1, DM], f32, tag="hbf")
            nc.sync.dma_start(out=hb_f[:, :], in_=hb_d[:, :])
            hb_bf = pers.tile([1, DM], bf16, tag="hbb")
            nc.vector.tensor_copy(hb_bf[:, :], hb_f[:, :])

            wo_bf = []
            for it in range(4):
                wof = aux.tile([128, DM], f32, tag="wof")
                nc.sync.dma_start(out=wof[:, :], in_=wo_d[it * 128 : (it + 1) * 128, :])
                wob = pers.tile([128, DM], bf16, tag=f"wo{it}")
                nc.vector.tensor_copy(wob[:, :], wof[:, :])
                wo_bf.append(wob)

            # ---- phase 1: projections ----
            xT = []
            for d in range(8):
                t = ptmp.tile([128, N], f32r, tag=f"xT{d}")
                for cc in range(4):
                    nc.sync.dma_start(
                        out=t[:, cc * 512 : (cc + 1) * 512],
                        in_=xT_d[d * 128 : (d + 1) * 128, cc * 512 : (cc + 1) * 512],
                    )
                xT.append(t)

            kT = [pers.tile([128, N], bf16, tag=f"kT{i}", name=f"kT{i}") for i in range(4)]
            qT = [pers.tile([128, N], bf16, tag=f"qT{i}", name=f"qT{i}") for i in range(4)]

            for w_d, dst in ((wk_d, kT), (wq_d, qT)):
                for it in range(4):
                    wt = []
                    for d in range(8):
                        t = wstr.tile([128, 128], f32r, tag="wt", bufs=16)
                        nc.sync.dma_start(
                            out=t[:, :],
                            in_=w_d[d * 128 : (d + 1) * 128, it * 128 : (it + 1) * 128],
                        )
                        wt.append(t)
                    for tt in range(4):
                        ps = mmps.tile([128, 512], f32, tag="mm")
                        for d in range(8):
                            nc.tensor.matmul(
                                ps[:, :],
                                lhsT=wt[d][:, :],
                                rhs=xT[d][:, tt * 512 : (tt + 1) * 512],
                                start=(d == 0),
                                stop=(d == 7),
                            )
                        nc.vector.tensor_copy(
                            dst[it][:, tt * 512 : (tt + 1) * 512], ps[:, :]
                        )

            wv = []
            for d in range(8):
                t = ptmp.tile([128, LI], f32r, tag=f"wv{d}")
                nc.sync.dma_start(out=t[:, :], in_=wv_d[d * 128 : (d + 1) * 128, :])
                wv.append(t)

            v_aug = [pers.tile([128, HG * VW], bf16, tag=f"va{t}", name=f"va{t}") for t in range(16)]
            for tt in range(16):
                va3 = v_aug[tt].rearrange("p (h c) -> p h c", h=HG)
                nc.vector.memset(va3[:, :, DH : DH + 1], 1.0)
                ps = mmps.tile([128, 512], f32, tag="mm")
                for d in range(8):
                    nc.tensor.matmul(
                        ps[:, :],
                        lhsT=xT[d][:, tt * 128 : (tt + 1) * 128],
                        rhs=wv[d][:, :],
                        start=(d == 0),
                        stop=(d == 7),
                    )
                nc.vector.tensor_copy(
                    va3[:, :, 0:DH], ps.rearrange("p (h c) -> p h c", h=HG)
                )

            if dbg:
                for src_t, dst_d in ((kT[0], dkT), (qT[0], dqT)):
                    for pc in range(4):
                        dc = aux.tile([128, 512], f32, tag="dbgc", bufs=1)
                        nc.vector.tensor_copy(dc[:, :], src_t[:, pc * 512 : (pc + 1) * 512])
                        nc.sync.dma_start(out=dst_d[:, pc * 512 : (pc + 1) * 512], in_=dc[:, :])
                dc = aux.tile([128, HG * VW], f32, tag="dbgc2", bufs=1)
                nc.vector.tensor_copy(dc[:, :], v_aug[0][:, :])
                nc.sync.dma_start(out=dva[:, :], in_=dc[:, :])

            # ---- phases 2+3: attention + out-proj + RS, per q-chunk ----
            for c in range(NCHUNK):
                nk = 4 * (c + 1)
                aos = [work.tile([128, CHUNK], bf16, tag=f"ao{i}", name=f"ao{i}", bufs=2) for i in range(4)]
                for hp in range(4):
                    pvs = [pvps.tile([VW, CHUNK], f32, tag="pv", name="pv") for _ in range(2)]
                    sims_of = {}

                    def qk_step(jb):
                        sims = [
                            simps.tile([128, CHUNK], f32, tag="sim", name="sim")
                            for _ in range(2)
                        ]
                        for e in range(2):
                            nc.tensor.matmul(
                                sims[e][:, :],
                                lhsT=kT[hp][
                                    64 * e : 64 * e + 64, jb * KB : (jb + 1) * KB
                                ],
                                rhs=qT[hp][
                                    64 * e : 64 * e + 64, c * CHUNK : (c + 1) * CHUNK
                                ],
                                start=True,
                                stop=True,
                            )
                        sims_of[jb] = sims

                    def pv_step(jb):
                        sims = sims_of.pop(jb)
                        for e in range(2):
                            h = 2 * hp + e
                            pt = work.tile([128, CHUNK], bf16, tag=f"pt{e}", bufs=3)
                            nc.scalar.activation(
                                pt[:, :], sims[e][:, :], Exp, scale=float(DH**-0.5)
                            )
                            v = jb - (nk - 4)
                            if v >= 0:
                                nc.vector.tensor_mul(
                                    pt[:, :],
                                    pt[:, :],
                                    mask_sb[:, v * CHUNK : (v + 1) * CHUNK],
                                )
                            nc.tensor.matmul(
                                pvs[e][:, :],
                                lhsT=v_aug[jb][:, h * VW : (h + 1) * VW],
                                rhs=pt[:, :],
                                start=(jb == 0),
                                stop=(jb == nk - 1),
                            )

                    qk_step(0)
                    for jb in range(1, nk):
                        qk_step(jb)
                        pv_step(jb - 1)
                    pv_step(nk - 1)
                    if dbg and c == 0 and hp == 0:
                        dc = aux.tile([VW, CHUNK], f32, tag="dbgc4", bufs=1)
                        nc.vector.tensor_copy(dc[:, :], pvs[0][:, :])
                        nc.sync.dma_start(out=dpv[:, :], in_=dc[:, :])
                    for e in range(2):
                        recip = work.tile([1, CHUNK], f32, tag="recip", bufs=2)
                        nc.vector.reciprocal(recip[:, :], pvs[e][DH : DH + 1, :])
                        rb = work.tile([64, CHUNK], f32, tag="rb", bufs=2)
                        rsrc = bass.AP(
                            tensor=recip.tensor,
                            offset=recip.offset,
                            ap=[[1, 1], [0, 64], [1, CHUNK]],
                        )
                        nc.sync.dma_start(out=rb[:, :], in_=rsrc)
                        nc.vector.tensor_mul(
                            aos[hp][64 * e : 64 * e + 64, :], pvs[e][0:DH, :], rb[:, :]
                        )

                if dbg and c == 0:
                    dc = aux.tile([128, CHUNK], f32, tag="dbgc5", bufs=1)
                    nc.vector.tensor_copy(dc[:, :], aos[0][:, :])
                    nc.sync.dma_start(out=dao[:, :], in_=dc[:, :])
                pd = dram.tile([CHUNK, DM], f32, tag="pd")
                for ts in range(4):
                    for ct in range(2):
                        po = mmps.tile([128, 512], f32, tag="mm")
                        nc.tensor.matmul(
                            po[:, :],
                            lhsT=ones_bf[:, :],
                            rhs=hb_bf[:, ct * 512 : (ct + 1) * 512],
                            start=True,
                            stop=False,
                        )
                        for it in range(4):
                            nc.tensor.matmul(
                                po[:, :],
                                lhsT=aos[it][:, ts * 128 : (ts + 1) * 128],
                                rhs=wo_bf[it][:, ct * 512 : (ct + 1) * 512],
                                start=False,
                                stop=(it == 3),
                            )
                        ob = work.tile([128, 512], f32, tag="ob", name="ob", bufs=2)
                        nc.vector.tensor_copy(ob[:, :], po[:, :])
                        nc.sync.dma_start(
                            out=pd[ts * 128 : (ts + 1) * 128, ct * 512 : (ct + 1) * 512],
                            in_=ob[:, :],
                        )

                if dbg and c == 0:
                    nc.sync.dma_start(out=dpd[:, :], in_=pd[:, :])
                rs = dram.tile([CHUNK // 2, DM], f32, tag="rs")
                nc.gpsimd.collective_compute(
                    "ReduceScatter",
                    mybir.AluOpType.add,
                    replica_groups=RG,
                    ins=[pd.opt()],
                    outs=[rs.opt()],
                )
                nc.sync.dma_start(
                    out=out_d[c * 256 : (c + 1) * 256, :], in_=rs[:, :]
                )

    nc.finalize()
    return nc


def _get_graph():
    global _GRAPH
    if _GRAPH is None:
        _GRAPH = _build_graph()
    return _GRAPH


def _build_masks():
    m = np.zeros((KB, 4, 4, KB), np.float32)
    tri = np.triu(np.ones((KB, KB), np.float32))  # [j, ti] = 1 where ti >= j
    for v in range(4):
        for qi in range(4):
            if qi > v:
                m[:, v, qi, :] = 1.0
            elif qi == v:
                m[:, v, qi, :] = tri
    return np.ascontiguousarray(m.reshape(KB, 4 * CHUNK))


def _make_in_maps(x, w_qkv, w_out, b_out):
    x = np.asarray(x, np.float32)
    w_qkv = np.asarray(w_qkv, np.float32)
    w_out = np.asarray(w_out, np.float32)
    b_out = np.asarray(b_out, np.float32)
    import ml_dtypes

    xT = [np.ascontiguousarray(x[b].T) for b in range(B)]
    masks = _build_masks().astype(ml_dtypes.bfloat16)
    hb = np.ascontiguousarray((0.5 * b_out).reshape(1, DM))
    in_maps = []
    for c in range(NCORES):
        b, g = c // 2, c % 2
        in_maps.append(
            {
                "xT": xT[b],
                "wq": np.ascontiguousarray(w_qkv[:, LI * g : LI * (g + 1)]),
                "wk": np.ascontiguousarray(w_qkv[:, DM + LI * g : DM + LI * (g + 1)]),
                "wv": np.ascontiguousarray(
                    w_qkv[:, 2 * DM + LI * g : 2 * DM + LI * (g + 1)]
                ),
                "wo": np.ascontiguousarray(w_out[LI * g : LI * (g + 1), :]),
                "hb": hb,
                "mask": masks,
            }
        )
    return in_maps


def _assemble(results):
    y = np.empty((B, N, DM), np.float32)
    for c in range(NCORES):
        b, g = c // 2, c % 2
        o = results[c]["out"]  # [1024, 1024]: 4 stripes of 256 tokens
        for ch in range(NCHUNK):
            t0 = ch * CHUNK + g * 256
            y[b, t0 : t0 + 256] = o[ch * 256 : (ch + 1) * 256]
    return y


def _install_ntff_hook_shim():
    """The container's antenv package lacks axon_hooks; synthesize it so
    run_bass_kernel_spmd(trace=True) can NTFF-profile via the injected .so."""
    import types

    if "antenv.axon_hooks" in sys.modules:
        return
    try:
        from trn_agent_boot.trn_boot import _ntff_profile_via_ctypes

        hook = _ntff_profile_via_ctypes("/opt/axon/libaxon_pjrt.so")
    except Exception as e:  # profiling degrades, run still works
        print(f"ntff hook shim unavailable: {e}")
        hook = None
    mod = types.ModuleType("antenv.axon_hooks")
    _state = {"hook": hook}
    mod.set_axon_ntff_profile_hook = lambda h: _state.__setitem__("hook", h)
    mod.get_axon_ntff_profile_hook = lambda: _state["hook"]
    sys.modules["antenv.axon_hooks"] = mod
    import antenv

    antenv.axon_hooks = mod


def _run(in_maps, trace=False):
    from concourse import bass_utils

    if trace:
        _install_ntff_hook_shim()
    nc = _get_graph()
    return bass_utils.run_bass_kernel_spmd(
        nc, in_maps, core_ids=list(range(NCORES)), trace=trace
    )


def kernel(x, w_qkv, w_out, b_out):
    res = _run(_make_in_maps(x, w_qkv, w_out, b_out), trace=False)
    return _assemble(res.results)


def kernel_timed(x, w_qkv, w_out, b_out):
    res = _run(_make_in_maps(x, w_qkv, w_out, b_out), trace=True)
    return _assemble(res.results), res
